# revision 22
# baseline (speedup 1.0000x reference)
"""Trainium2 Bass kernel for nn_AttentionLayer (B=64, F=1024, K=1024).

Reference computation (per batch b):
    scores[k, g] = sum_f input[b, f, k] * weight[f, g] + bias[g]
    alpha        = softmax(scores, axis=g)
    out[b, f, k] = input[b, f, k] * alpha[k, f]

Strategy: data-parallel over batch across 8 NeuronCores (8 batches/core).
Per batch, everything is computed in the transposed [g, k] layout so that no
transposes are ever needed:
    scoresT[g, k] = sum_f W[f, g] * X[f, k]      (lhsT = W chunk, rhs = X chunk)
    E[g, k]  = exp(scoresT + bias[g])            (ScalarE, bias is per-partition,
                                                  output bf16)
    T[g, k]  = sum over the 8 g-chunk tiles      (7 bf16 VectorE adds, hidden
                                                  under the matmuls)
    S[., k]  = sum_g T[g, k]                     (ONE matmul with ones[128,128]
                                                  stationary -> sum replicated
                                                  across partitions = free bcast;
                                                  GPSIMD partition_all_reduce
                                                  measured 11us slower)
    D = 1/S  (reciprocal_approx_fast, fp32) ->cast to bf16 on ScalarE
    out[f, k] = X[f, k] * E[f, k] * D[k]         (VectorE, g === f axis)

All tensor data is bf16 (inputs converted host-side, output upcast host-side);
only PSUM accumulation, bias and the reciprocal run in fp32.  bf16 matmuls run
at the same 1 cycle/row as float32r, but bf16 halves DMA traffic (34 MB/core
vs 68) and doubles VectorE throughput (2x_1P perf mode), so the PE becomes the
sole bottleneck.  The work is software-pipelined over half-batch "slabs"
(k split in two); the partition-sum matmul + epilogue of slab N-1 is emitted
AFTER the first two main matmul groups of slab N, so the PE stream has no
dependency stall at slab boundaries (by the time groups 0-1 of slab N finish,
T[N-1] has long been accumulated).  Max rel err vs the fp32 reference ~7e-3.
"""

import sys
from contextlib import ExitStack

import numpy as np

for _p in ("/opt/trn_rl_repo", "/root/.axon_site/_ro/trn_rl_repo"):
    if _p not in sys.path:
        sys.path.append(_p)

import concourse.bacc as bacc
import concourse.bass as bass
import concourse.bass_isa as bass_isa
import concourse.mybir as mybir
import concourse.tile as tile
from concourse.bass_utils import run_bass_kernel_spmd

N_CORES = 8
B, F, K = 64, 1024, 1024
BPC = B // N_CORES            # batches per core
P = 128                       # SBUF partitions
NF = F // P                   # f (contraction) chunks
NG = F // P                   # g (feature/output-partition) chunks
KC = 512                      # moving free-dim chunk (one PSUM bank in fp32)
NK = K // KC
PRE = 2                       # main matmul groups emitted before prev epilogue

FP32 = mybir.dt.float32
BF16 = mybir.dt.bfloat16

EXP = mybir.ActivationFunctionType.Exp


def _build(bpc: int = BPC, reps: int = 1):
    nc = bacc.Bacc("TRN2", target_bir_lowering=False, debug=False)

    x_d = nc.dram_tensor("x", [bpc, F, K], BF16, kind="ExternalInput").ap()
    w_d = nc.dram_tensor("w", [F, F], BF16, kind="ExternalInput").ap()
    b_d = nc.dram_tensor("b", [1, F], FP32, kind="ExternalInput").ap()
    o_d = nc.dram_tensor("out", [bpc, F, K], BF16, kind="ExternalOutput").ap()

    with tile.TileContext(nc) as tc, ExitStack() as ctx:
        w_pool = ctx.enter_context(tc.tile_pool(name="w", bufs=1))
        c_pool = ctx.enter_context(tc.tile_pool(name="const", bufs=1))
        x_pool = ctx.enter_context(tc.tile_pool(name="x", bufs=16))
        e_pool = ctx.enter_context(tc.tile_pool(name="e", bufs=12))
        p_pool = ctx.enter_context(tc.tile_pool(name="pp", bufs=12))
        t_pool = ctx.enter_context(tc.tile_pool(name="t", bufs=3))
        d_pool = ctx.enter_context(tc.tile_pool(name="d", bufs=3))
        db_pool = ctx.enter_context(tc.tile_pool(name="db", bufs=3))
        o_pool = ctx.enter_context(tc.tile_pool(name="o", bufs=6))
        sc_psum = ctx.enter_context(tc.tile_pool(name="sc", bufs=6, space="PSUM"))
        s_psum = ctx.enter_context(tc.tile_pool(name="s", bufs=2, space="PSUM"))

        # ---- constants; emitted inside the first prefetch, AFTER batch-0's
        # critical chunks (bias/ones aren't read until well into slab 0)
        bias_sb = c_pool.tile([P, NG], FP32)
        ones_sb = c_pool.tile([P, P], BF16)

        def load_consts():
            nc.sync.dma_start(
                out=bias_sb[:], in_=b_d.rearrange("o (c p) -> (o p) c", p=P)
            )
            nc.vector.memset(ones_sb[:], 1.0)

        # w_sb[p, fc*F + g] = W[fc*128 + p, g]
        w_sb = w_pool.tile([P, NF * F], BF16)

        def w_tile(fc, gc):
            off = fc * F + gc * P
            return w_sb[:, off : off + P]

        def prefetch_x(b, with_w=False):
            # 8 separate chunk tiles + 8 separate DMAs: spreads the loads
            # across DMA queues and lets the first matmul groups start as
            # soon as their own chunk lands (a single merged strided DMA
            # measured 9us slower)
            x_tiles = []
            for fc in range(NF):
                x_t = x_pool.tile([P, K], BF16, tag="x")
                if with_w:
                    # startup: interleave the g-low half of W with batch-0's
                    # kc=0 X halves so slab 0's first matmul groups start as
                    # early as possible; the rest streams in behind them
                    nc.sync.dma_start(
                        out=w_sb[:, fc * F : fc * F + F // 2],
                        in_=w_d[fc * P : (fc + 1) * P, 0 : F // 2],
                    )
                    nc.sync.dma_start(
                        out=x_t[:, 0:KC], in_=x_d[b, fc * P : (fc + 1) * P, 0:KC]
                    )
                else:
                    nc.sync.dma_start(
                        out=x_t[:], in_=x_d[b, fc * P : (fc + 1) * P, :]
                    )
                x_tiles.append(x_t)
            if with_w:
                load_consts()
                for fc in range(NF):
                    nc.sync.dma_start(
                        out=w_sb[:, fc * F + F // 2 : (fc + 1) * F],
                        in_=w_d[fc * P : (fc + 1) * P, F // 2 : F],
                    )
                for fc in range(NF):
                    nc.sync.dma_start(
                        out=x_tiles[fc][:, KC:K],
                        in_=x_d[b, fc * P : (fc + 1) * P, KC:K],
                    )
            return x_tiles

        def xs(x_tiles, fc, kc):
            return x_tiles[fc][:, kc * KC : (kc + 1) * KC]

        def emit_group(kc, x_tiles, gc, st):
            """One main matmul group (8 accumulating matmuls) + exp + partial
            E-sum + P = X*E for chunk gc of one (batch, k-half) slab.

            The adds and muls run on VectorE in bf16 (2x_1P mode) in the
            shadow of the matmuls."""
            sc = sc_psum.tile([P, KC], FP32, tag="sc")
            for fc in range(NF):
                nc.tensor.matmul(
                    sc[:],
                    lhsT=w_tile(fc, gc),
                    rhs=xs(x_tiles, fc, kc),
                    start=(fc == 0),
                    stop=(fc == NF - 1),
                )
            e_t = e_pool.tile([P, KC], BF16, tag="e")
            nc.scalar.activation(
                e_t[:], sc[:], EXP, bias=bias_sb[:, gc : gc + 1], scale=1.0
            )
            if gc == 1:
                st["t"] = t_pool.tile([P, KC], BF16, tag="t", name="t_t")
                nc.vector.tensor_add(st["t"][:], st["e0"][:], e_t[:])
            elif gc > 1:
                nc.vector.tensor_add(st["t"][:], st["t"][:], e_t[:])
            else:
                st["e0"] = e_t
            # P = X * E needs no denominator -> runs in the shadow of the
            # matmuls, leaving only P * (1/S) for the slab epilogue
            p_t = p_pool.tile([P, KC], BF16, tag="pp")
            nc.vector.tensor_mul(p_t[:], xs(x_tiles, gc, kc), e_t[:])
            st["p"].append(p_t)

        def slab_out(b, kc, p_tiles, t_t, split=1):
            """Partition-sum matmul + reciprocal + final scale + DMA out.

            Emitted after PRE main groups of the NEXT slab: by then t_t is
            complete, so the single ones-matmul never stalls the PE.
            split>1 pipelines the epilogue in k-fractions (used for the very
            last slab, where nothing else overlaps the recip->mul->DMA chain).
            """
            s_t = s_psum.tile([P, KC], FP32, tag="s")
            nc.tensor.matmul(
                s_t[:], lhsT=ones_sb[:], rhs=t_t[:], start=True, stop=True
            )
            qw = KC // split
            for q in range(split):
                qs = slice(q * qw, (q + 1) * qw)
                d_t = d_pool.tile([P, qw], FP32, tag="d")
                nc.vector.reciprocal_approx_fast(d_t[:], s_t[:, qs])
                d_b = db_pool.tile([P, qw], BF16, tag="db")
                nc.scalar.copy(d_b[:], d_t[:])
                for fc in range(NF):
                    o_t = o_pool.tile([P, qw], BF16, tag="o")
                    nc.vector.tensor_mul(o_t[:], p_tiles[fc][:, qs], d_b[:])
                    nc.sync.dma_start(
                        out=o_d[
                            b,
                            fc * P : (fc + 1) * P,
                            kc * KC + q * qw : kc * KC + (q + 1) * qw,
                        ],
                        in_=o_t[:],
                    )

        # software pipeline over half-batch slabs: slab s-1's epilogue
        # (partition-sum on Pool, recip + muls on DVE, DMA out) is emitted
        # after the first PRE main matmul groups of slab s, so it overlaps
        # slab s's matmuls and the PE stream never waits on it.
        prev = None
        first = True
        for _ in range(reps):
            for b in range(bpc):
                x_tiles = prefetch_x(b, with_w=first)
                first = False
                for kc in range(NK):
                    st = {"p": [], "t": None, "e0": None}
                    for gc in range(PRE):
                        emit_group(kc, x_tiles, gc, st)
                    if prev is not None:
                        slab_out(*prev)
                    for gc in range(PRE, NG):
                        emit_group(kc, x_tiles, gc, st)
                    prev = (b, kc, st["p"], st["t"])
        slab_out(*prev, split=4)

    nc.compile()
    return nc


_NC = None


def _get_nc():
    global _NC
    if _NC is None:
        _NC = _build()
    return _NC


def _bf16(a):
    import ml_dtypes

    return np.ascontiguousarray(np.asarray(a, dtype=np.float32)).astype(
        ml_dtypes.bfloat16
    )


def build_in_maps(inputs):
    x = _bf16(inputs["input"])
    w = _bf16(inputs["weight"])
    b = np.ascontiguousarray(np.asarray(inputs["bias"], dtype=np.float32))
    return [
        {"x": x[c * BPC : (c + 1) * BPC], "w": w, "b": b}
        for c in range(N_CORES)
    ]


def kernel(**inputs) -> np.ndarray:
    nc = _get_nc()
    in_maps = build_in_maps(inputs)
    res = run_bass_kernel_spmd(nc, in_maps, list(range(N_CORES)))
    return np.concatenate(
        [res.results[c]["out"].astype(np.float32) for c in range(N_CORES)], axis=0
    )


# revision 26
# speedup vs baseline: 1.0039x; 1.0039x over previous
"""Trainium2 Bass kernel for nn_AttentionLayer (B=64, F=1024, K=1024).

Reference computation (per batch b):
    scores[k, g] = sum_f input[b, f, k] * weight[f, g] + bias[g]
    alpha        = softmax(scores, axis=g)
    out[b, f, k] = input[b, f, k] * alpha[k, f]

Strategy: data-parallel over batch across 8 NeuronCores (8 batches/core).
Per batch, everything is computed in the transposed [g, k] layout so that no
transposes are ever needed:
    scoresT[g, k] = sum_f W[f, g] * X[f, k]      (lhsT = W chunk, rhs = X chunk)
    E[g, k]  = exp(scoresT + bias[g])            (ScalarE, bias is per-partition,
                                                  output bf16)
    T[g, k]  = sum over the 8 g-chunk tiles      (7 bf16 VectorE adds, hidden
                                                  under the matmuls)
    S[., k]  = sum_g T[g, k]                     (ONE matmul with ones[128,128]
                                                  stationary -> sum replicated
                                                  across partitions = free bcast;
                                                  GPSIMD partition_all_reduce
                                                  measured 11us slower)
    D = 1/S  (reciprocal_approx_fast, fp32) ->cast to bf16 on ScalarE
    out[f, k] = X[f, k] * E[f, k] * D[k]         (VectorE, g === f axis)

All tensor data is bf16 (inputs converted host-side, output upcast host-side);
only PSUM accumulation, bias and the reciprocal run in fp32.  bf16 matmuls run
at the same 1 cycle/row as float32r, but bf16 halves DMA traffic (34 MB/core
vs 68) and doubles VectorE throughput (2x_1P perf mode), so the PE becomes the
sole bottleneck.  The work is software-pipelined over half-batch "slabs"
(k split in two); the partition-sum matmul + epilogue of slab N-1 is emitted
AFTER the first two main matmul groups of slab N, so the PE stream has no
dependency stall at slab boundaries (by the time groups 0-1 of slab N finish,
T[N-1] has long been accumulated).  Max rel err vs the fp32 reference ~7e-3.
"""

import sys
from contextlib import ExitStack

import numpy as np

for _p in ("/opt/trn_rl_repo", "/root/.axon_site/_ro/trn_rl_repo"):
    if _p not in sys.path:
        sys.path.append(_p)

import concourse.bacc as bacc
import concourse.bass as bass
import concourse.bass_isa as bass_isa
import concourse.mybir as mybir
import concourse.tile as tile
from concourse.bass_utils import run_bass_kernel_spmd

N_CORES = 8
B, F, K = 64, 1024, 1024
BPC = B // N_CORES            # batches per core
P = 128                       # SBUF partitions
NF = F // P                   # f (contraction) chunks
NG = F // P                   # g (feature/output-partition) chunks
KC = 512                      # moving free-dim chunk (one PSUM bank in fp32)
NK = K // KC
PRE = 2                       # main matmul groups emitted before prev epilogue

FP32 = mybir.dt.float32
BF16 = mybir.dt.bfloat16

EXP = mybir.ActivationFunctionType.Exp


def _build(bpc: int = BPC, reps: int = 1):
    nc = bacc.Bacc("TRN2", target_bir_lowering=False, debug=False)

    x_d = nc.dram_tensor("x", [bpc, F, K], BF16, kind="ExternalInput").ap()
    w_d = nc.dram_tensor("w", [F, F], BF16, kind="ExternalInput").ap()
    b_d = nc.dram_tensor("b", [1, F], FP32, kind="ExternalInput").ap()
    o_d = nc.dram_tensor("out", [bpc, F, K], BF16, kind="ExternalOutput").ap()

    with tile.TileContext(nc) as tc, ExitStack() as ctx:
        w_pool = ctx.enter_context(tc.tile_pool(name="w", bufs=1))
        c_pool = ctx.enter_context(tc.tile_pool(name="const", bufs=1))
        x_pool = ctx.enter_context(tc.tile_pool(name="x", bufs=16))
        e_pool = ctx.enter_context(tc.tile_pool(name="e", bufs=12))
        p_pool = ctx.enter_context(tc.tile_pool(name="pp", bufs=12))
        t_pool = ctx.enter_context(tc.tile_pool(name="t", bufs=3))
        d_pool = ctx.enter_context(tc.tile_pool(name="d", bufs=3))
        db_pool = ctx.enter_context(tc.tile_pool(name="db", bufs=3))
        o_pool = ctx.enter_context(tc.tile_pool(name="o", bufs=6))
        sc_psum = ctx.enter_context(tc.tile_pool(name="sc", bufs=6, space="PSUM"))
        s_psum = ctx.enter_context(tc.tile_pool(name="s", bufs=2, space="PSUM"))

        # ---- constants; emitted inside the first prefetch, AFTER batch-0's
        # critical chunks (bias/ones aren't read until well into slab 0)
        bias_sb = c_pool.tile([P, NG], FP32)
        ones_sb = c_pool.tile([P, P], BF16)

        def load_consts():
            nc.sync.dma_start(
                out=bias_sb[:], in_=b_d.rearrange("o (c p) -> (o p) c", p=P)
            )
            nc.vector.memset(ones_sb[:], 1.0)

        # w_sb[p, fc*F + g] = W[fc*128 + p, g]
        w_sb = w_pool.tile([P, NF * F], BF16)

        def w_tile(fc, gc):
            off = fc * F + gc * P
            return w_sb[:, off : off + P]

        def prefetch_x(b, with_w=False):
            # 8 separate chunk tiles + 8 separate DMAs: spreads the loads
            # across DMA queues and lets the first matmul groups start as
            # soon as their own chunk lands (a single merged strided DMA
            # measured 9us slower)
            x_tiles = []
            for fc in range(NF):
                x_t = x_pool.tile([P, K], BF16, tag="x")
                if with_w:
                    # startup: interleave the g-low half of W with batch-0's
                    # kc=0 X halves so slab 0's first matmul groups start as
                    # early as possible; the rest streams in behind them
                    # (per-g-chunk W loads measured slower: 256B/partition
                    # descriptors halve DMA efficiency)
                    nc.sync.dma_start(
                        out=w_sb[:, fc * F : fc * F + F // 2],
                        in_=w_d[fc * P : (fc + 1) * P, 0 : F // 2],
                    )
                    nc.sync.dma_start(
                        out=x_t[:, 0:KC], in_=x_d[b, fc * P : (fc + 1) * P, 0:KC]
                    )
                else:
                    nc.sync.dma_start(
                        out=x_t[:], in_=x_d[b, fc * P : (fc + 1) * P, :]
                    )
                x_tiles.append(x_t)
            if with_w:
                load_consts()
                for fc in range(NF):
                    nc.sync.dma_start(
                        out=w_sb[:, fc * F + F // 2 : (fc + 1) * F],
                        in_=w_d[fc * P : (fc + 1) * P, F // 2 : F],
                    )
                for fc in range(NF):
                    nc.sync.dma_start(
                        out=x_tiles[fc][:, KC:K],
                        in_=x_d[b, fc * P : (fc + 1) * P, KC:K],
                    )
            return x_tiles

        def emit_group(koff, kw, x_tiles, gc, st):
            """One main matmul group (8 accumulating matmuls) + exp + partial
            E-sum + P = X*E for chunk gc of one (batch, k-slab) slab.

            The adds and muls run on VectorE in bf16 (2x_1P mode) in the
            shadow of the matmuls."""
            ks = slice(koff, koff + kw)
            sc = sc_psum.tile([P, kw], FP32, tag="sc")
            for fc in range(NF):
                nc.tensor.matmul(
                    sc[:],
                    lhsT=w_tile(fc, gc),
                    rhs=x_tiles[fc][:, ks],
                    start=(fc == 0),
                    stop=(fc == NF - 1),
                )
            e_t = e_pool.tile([P, kw], BF16, tag="e")
            nc.scalar.activation(
                e_t[:], sc[:], EXP, bias=bias_sb[:, gc : gc + 1], scale=1.0
            )
            if gc == 1:
                st["t"] = t_pool.tile([P, kw], BF16, tag="t", name="t_t")
                nc.vector.tensor_add(st["t"][:], st["e0"][:], e_t[:])
            elif gc > 1:
                nc.vector.tensor_add(st["t"][:], st["t"][:], e_t[:])
            else:
                st["e0"] = e_t
            # P = X * E needs no denominator -> runs in the shadow of the
            # matmuls, leaving only P * (1/S) for the slab epilogue
            p_t = p_pool.tile([P, kw], BF16, tag="pp")
            nc.vector.tensor_mul(p_t[:], x_tiles[gc][:, ks], e_t[:])
            st["p"].append(p_t)

        def slab_out(b, koff, kw, p_tiles, t_t, cast=True):
            """Partition-sum matmul + reciprocal + final scale + DMA out.

            Emitted after PRE main groups of the NEXT slab: by then t_t is
            complete, so the single ones-matmul never stalls the PE.
            cast=False skips the Act-engine bf16 cast of 1/S and multiplies
            with fp32 directly (used on the very last sub-slab, where the
            round trip to ScalarE would lengthen the pipeline-drain tail).
            """
            s_t = s_psum.tile([P, kw], FP32, tag="s")
            nc.tensor.matmul(
                s_t[:], lhsT=ones_sb[:], rhs=t_t[:], start=True, stop=True
            )
            d_t = d_pool.tile([P, kw], FP32, tag="d")
            nc.vector.reciprocal_approx_fast(d_t[:], s_t[:])
            if cast:
                d_b = db_pool.tile([P, kw], BF16, tag="db")
                nc.scalar.copy(d_b[:], d_t[:])
            else:
                d_b = d_t
            for fc in range(NF):
                o_t = o_pool.tile([P, kw], BF16, tag="o")
                nc.vector.tensor_mul(o_t[:], p_tiles[fc][:], d_b[:])
                nc.sync.dma_start(
                    out=o_d[b, fc * P : (fc + 1) * P, koff : koff + kw],
                    in_=o_t[:],
                )

        # software pipeline over half-batch slabs: slab s-1's epilogue
        # (sum-matmul, recip + muls on DVE, DMA out) is emitted after the
        # first PRE main matmul groups of slab s, so it overlaps slab s's
        # matmuls and the PE stream never waits on it.  The very last batch
        # tapers into smaller sub-slabs (512/256/128/128) so the epilogue
        # chain left after the final matmul -- the pipeline-drain tail of a
        # single execution -- covers 128 columns instead of 512.
        steady = [(kc * KC, KC) for kc in range(NK)]
        taper = [(0, 512), (512, 256), (768, 128), (896, 128)]
        prev = None
        first = True
        for r in range(reps):
            for b in range(bpc):
                x_tiles = prefetch_x(b, with_w=first)
                first = False
                last_b = r == reps - 1 and b == bpc - 1
                for koff, kw in taper if last_b else steady:
                    st = {"p": [], "t": None, "e0": None}
                    for gc in range(PRE):
                        emit_group(koff, kw, x_tiles, gc, st)
                    if prev is not None:
                        slab_out(*prev)
                    for gc in range(PRE, NG):
                        emit_group(koff, kw, x_tiles, gc, st)
                    prev = (b, koff, kw, st["p"], st["t"])
        slab_out(*prev, cast=False)

    nc.compile()
    return nc


_NC = None


def _get_nc():
    global _NC
    if _NC is None:
        _NC = _build()
    return _NC


def _bf16(a):
    import ml_dtypes

    return np.ascontiguousarray(np.asarray(a, dtype=np.float32)).astype(
        ml_dtypes.bfloat16
    )


def build_in_maps(inputs):
    x = _bf16(inputs["input"])
    w = _bf16(inputs["weight"])
    b = np.ascontiguousarray(np.asarray(inputs["bias"], dtype=np.float32))
    return [
        {"x": x[c * BPC : (c + 1) * BPC], "w": w, "b": b}
        for c in range(N_CORES)
    ]


def kernel(**inputs) -> np.ndarray:
    nc = _get_nc()
    in_maps = build_in_maps(inputs)
    res = run_bass_kernel_spmd(nc, in_maps, list(range(N_CORES)))
    return np.concatenate(
        [res.results[c]["out"].astype(np.float32) for c in range(N_CORES)], axis=0
    )
